# revision 1
# baseline (speedup 1.0000x reference)
"""Average Hausdorff loss on 8 Trainium2 NeuronCores — banded/streamed KNN.

Host (numpy): edge detection, coordinate compaction, half-res EDT for
certified NN-distance upper bounds, per-tile pred *bands* (contiguous
index intervals guaranteed to contain all NN candidates both ways).
Bands are split to <=1024 cols, rank-matched across the 8 cores (sorted
by width; width at rank k = max over cores), and the rhs operand is
PRE-GATHERED per core into a position-packed schedule array, so the
device program has only compile-time offsets while every core computes
its own (tight) bands.

Device (raw Bass, SPMD over 8 cores, 2 pair-slots per core):
  PE : per job, matmuls of 6-row augmented operands over its W_k band
       -> PSUM = -(d^2)/4 exactly (two jobs per PSUM bank-group)
  ACT: one activation Copy (scale 2^-12) per PSUM group -> fp16 ring
  DVE: two batched fold ops per 4-job group (gth->pred NN partials)
  DMA: fp16 blocks stream to DRAM per group (pred->gth NN finished as a
       128-way column max on host), dg partials stream via GPSIMD queue
Host: column maxes, scatter-max into pred space, sqrt, means, nanmean.

Pads use a far sentinel coordinate so they always lose the max.
"""

import numpy as np

H = 256
W_IMG = 256
BC = 16
N_CORES = 8
SLOTS = 1
G_TILE = 128
QUANT = 32
W_CAP = 1024     # max job width (2 jobs <= 2048 fp32 = 4 PSUM banks)
FOLD_B = 4       # jobs per DVE fold group
NB = 6           # d2s ring depth (fold-group slots)
DVE_COPY_MOD = 10**9  # disabled: every Nth psum group's PSUM->SBUF copy runs on DVE
SENT = 16384.0
D2_SCALE = 2.0 ** -12
D2_BACK = -4.0 * 4096.0
EDT_SLACK = 0.01


def _edge_maps(x):
    m = x > 0.5
    p = np.pad(m, ((0, 0), (1, 1), (1, 1)), constant_values=True)
    e = np.ones_like(m)
    for dy in range(3):
        for dx in range(3):
            e &= p[:, dy:dy + H, dx:dx + W_IMG]
    return m & ~e


def _edt_full(mask):
    """Exact EDT of `mask` ([256,256] bool) by two separable min passes."""
    BIG = np.float32(1e9)
    col = np.where(mask, np.float32(0.0), BIG)
    ar = np.arange(256, dtype=np.float32)
    d2 = (ar[:, None] - ar[None, :]) ** 2
    D1 = np.empty((256, 256), np.float32)
    D2 = np.empty((256, 256), np.float32)
    for c0 in range(0, 256, 64):
        D1[:, c0:c0 + 64] = (d2[:, :, None] + col[None, :, c0:c0 + 64]).min(1)
    for r0 in range(0, 256, 64):
        D2[r0:r0 + 64] = (D1[r0:r0 + 64, None, :] + d2[None, :, :]).min(2)
    return np.sqrt(D2)


def _nn_upper_bound(edt_other, ys, xs):
    return edt_other[ys, xs] + EDT_SLACK


def _aug_g(cy, cx):
    n = cy.shape[0]
    out = np.zeros((6, n), np.float32)
    sq = cy * cy + cx * cx
    b1 = np.floor(sq / 256.0)
    b0 = sq - b1 * 256.0
    out[0] = cy * 0.5
    out[1] = cx * 0.5
    out[2] = -b1
    out[3] = -b0
    out[4] = -64.0
    out[5] = -0.25
    return out


def _aug_p(cy, cx):
    n = cy.shape[0]
    out = np.zeros((6, n), np.float32)
    sq = cy * cy + cx * cx
    b1 = np.floor(sq / 256.0)
    b0 = sq - b1 * 256.0
    out[0] = cy
    out[1] = cx
    out[2] = 64.0
    out[3] = 0.25
    out[4] = b1
    out[5] = b0
    return out


def _kd_tiles(gy, gx, T):
    """Split gth points into T spatially-local tiles of <=128 points
    (recursive median bisection, alternating axes)."""
    leaves = []

    def split(ids, nt, axis):
        if nt == 1:
            leaves.append(ids)
            return
        t1 = nt // 2
        keys = (gy[ids], gx[ids])[axis]
        order = np.argsort(keys, kind='stable')
        cut = (len(ids) * t1) // nt
        split(ids[order[:cut]], t1, 1 - axis)
        split(ids[order[cut:]], nt - t1, 1 - axis)

    split(np.arange(len(gy)), T, 0)
    return leaves


def _tile_reqs(tiles, gy, gx, py, px, u_g, v_p):
    """Per tile: sorted array of pred indices that (a) could be the NN of
    a tile point (certificate box) or (b) could have their NN in the tile
    (coverage box)."""
    reqs = []
    for ids in tiles:
        ymin, ymax = gy[ids].min(), gy[ids].max()
        xmin, xmax = gx[ids].min(), gx[ids].max()
        U = u_g[ids].max()
        V = v_p.max() if len(v_p) else 0.0
        # prefilter with the tile box, then refine per point
        cand = np.nonzero(
            (py >= ymin - max(U, V)) & (py <= ymax + max(U, V))
            & (px >= xmin - max(U, V)) & (px <= xmax + max(U, V)))[0]
        if len(cand) == 0:
            reqs.append(cand)
            continue
        cy, cx, cv = py[cand], px[cand], v_p[cand]
        ty, tx, tu = gy[ids], gx[ids], u_g[ids]
        dd = ((cy[None, :] - ty[:, None]).astype(np.float32) ** 2
              + (cx[None, :] - tx[:, None]).astype(np.float32) ** 2)
        # (a) certificate: pred within a tile point's u-disc
        # (b) coverage: tile point within the pred's v-disc
        hit = (dd <= (tu[:, None] ** 2)).any(0)
        hit |= (dd <= (cv[None, :] ** 2)).any(0)
        reqs.append(cand[np.nonzero(hit)[0]])
    return reqs


def _pair_bands(gy, gx, py, px, u_g, v_p, T):
    n_g, n_p = len(gy), len(py)
    bands = []
    for t in range(T):
        a, b = (t * n_g) // T, ((t + 1) * n_g) // T
        if b <= a:
            bands.append((0, 1))
            continue
        ymin, ymax = gy[a:b].min(), gy[a:b].max()
        U = u_g[a:b].max()
        lo1 = np.searchsorted(py, ymin - U, 'left')
        hi1 = np.searchsorted(py, ymax + U, 'right')
        sel = (py + v_p >= ymin) & (py - v_p <= ymax)
        nz = np.nonzero(sel)[0]
        if len(nz):
            lo2, hi2 = nz[0], nz[-1] + 1
        else:
            lo2, hi2 = lo1, hi1
        lo, hi = int(min(lo1, lo2)), int(max(hi1, hi2))
        hi = max(hi, lo + 1)
        bands.append((lo, hi))
    return bands


def _pair_jobs(reqs):
    """Split per-tile pred index sets into jobs (tile, idx_chunk) of
    <=W_CAP points, sorted by quantized width desc."""
    jobs = []
    for t, r in enumerate(reqs):
        n = max(1, len(r))
        n_sp = -(-n // W_CAP)
        for c in range(n_sp):
            chunk = r[(c * n) // n_sp:((c + 1) * n) // n_sp]
            jobs.append((t, chunk))
    jobs.sort(key=lambda j: -len(j[1]))
    return jobs


def _job_w(job):
    return (-(-max(1, len(job[-1])) // QUANT)) * QUANT


def _plan_slot(jobs_8):
    """jobs_8: jobs list per pair of the slot.

    Packs width-desc ranks greedily into PSUM groups of <= 2048 columns
    (group members padded to the group max width).  Returns (widths,
    offsets, perm, groups) with groups = [(r0, nt, Wg)].
    """
    nrank = max(len(j) for j in jobs_8)
    widths = []
    for k in range(nrank):
        widths.append(max((_job_w(j[k]) for j in jobs_8 if len(j) > k),
                          default=QUANT))
    groups = []
    k = 0
    while k < nrank:
        Wg = widths[k]
        nt = min(2048 // Wg, nrank - k)
        for j in range(k, k + nt):
            widths[j] = Wg
        groups.append((k, nt, Wg))
        k += nt
    offs = np.concatenate([[0], np.cumsum(widths)]).astype(int)
    perm = list(range(nrank))
    return widths, offs, perm, groups


def _build_program(slot_w, slot_T, slot_groups):
    """slot_w: per slot, padded rank widths.  slot_T: gaug tiles per
    slot.  slot_groups: per slot, [(r0, nt, Wg)] PSUM groups."""
    from contextlib import ExitStack
    import concourse.bass as bass
    import concourse.mybir as mybir

    f32 = mybir.dt.float32
    f16 = mybir.dt.float16
    bf16 = mybir.dt.bfloat16

    nc = bass.Bass()
    C = [int(sum(w)) for w in slot_w]
    Cq = [c // 4 for c in C]
    TG = [slot_T[s] * G_TILE for s in range(SLOTS)]

    aug_d, dp_d = [], []
    for s in range(SLOTS):
        aug_d.append(nc.declare_dram_parameter(
            f"aug{s}", [6, TG[s] + C[s]], bf16, isOutput=False))
        dp_d.append(nc.declare_dram_parameter(
            f"dp{s}", [G_TILE, C[s]], f16, isOutput=True))

    groups = []   # (slot, r0, nt, Wg)
    for s in range(SLOTS):
        for (r0, nt, Wg) in slot_groups[s]:
            groups.append((s, r0, nt, Wg))
    G = len(groups)
    offs = [np.concatenate([[0], np.cumsum(w)]).astype(int) for w in slot_w]
    rank_tile = _build_program.rank_tile
    # input layout: [gaug group0 | paug group0 | gaug rest | paug rest]
    n0 = [slot_groups[s][0][1] for s in range(SLOTS)]
    g0w = [int(offs[s][n0[s]]) for s in range(SLOTS)]

    def goff(s, k):
        return k * G_TILE if k < n0[s] else g0w[s] + k * G_TILE

    def poff(s, k, c):
        return n0[s] * G_TILE + c if k < n0[s] else TG[s] + c

    with ExitStack() as ctx:
        aug = []
        for s in range(SLOTS):
            aug.append(ctx.enter_context(
                nc.sbuf_tensor(f"augs{s}", [6, TG[s] + C[s]], bf16)))
        pt = [ctx.enter_context(nc.psum_tensor(f"pt{i}", [G_TILE, 2048], f32))
              for i in range(2)]
        d2s = ctx.enter_context(
            nc.sbuf_tensor("d2s", [G_TILE, NB, 2048], f16))

        inA_sems = [ctx.enter_context(nc.semaphore(f"dma_inA{s}"))
                    for s in range(SLOTS)]
        inB_sems = [ctx.enter_context(nc.semaphore(f"dma_inB{s}"))
                    for s in range(SLOTS)]
        pe_sem = ctx.enter_context(nc.semaphore("pe_done"))
        act_sem = ctx.enter_context(nc.semaphore("act_done"))
        out_sem = ctx.enter_context(nc.semaphore("dma_out"))
        block = ctx.enter_context(nc.Block())

        # first input chunk = group0's gaug tiles + group0's columns
        splitc = [n0[s] * G_TILE + g0w[s] for s in range(SLOTS)]

        @block.sync
        def _(sync):
            for s in range(SLOTS):
                sync.dma_start(aug[s][:, 0:splitc[s]],
                               aug_d[s][:, 0:splitc[s]],
                               ).then_inc(inA_sems[s], 16)
            for s in range(SLOTS):
                sync.dma_start(aug[s][:, splitc[s]:],
                               aug_d[s][:, splitc[s]:],
                               ).then_inc(inB_sems[s], 16)
            # dp stream per group (dg is derived host-side from the
            # same raw blocks -- no separate fold output)
            for i, (s, r0, nt, Wg) in enumerate(groups):
                o0, o1 = int(offs[s][r0]), int(offs[s][r0 + nt])
                sync.wait_ge(act_sem, 2 * i + 2)
                sync.dma_start(dp_d[s][:, o0:o1],
                               d2s[:, i % NB, 0:nt * Wg],
                               ).then_inc(out_sem, 32)

        @block.tensor
        def _(tensor):
            cur_slot = -1
            waited_b = False
            for i, (s, r0, nt, Wg) in enumerate(groups):
                if s != cur_slot:
                    tensor.wait_ge(inA_sems[s], 16)
                    cur_slot = s
                    waited_b = False
                if not waited_b and r0 > 0:
                    tensor.wait_ge(inB_sems[s], 16)
                    waited_b = True
                if i >= 2:
                    tensor.wait_ge(act_sem, 2 * i - 2)
                half = nt // 2 if nt >= 4 else 0
                mm = None
                for j in range(nt):
                    k = r0 + j
                    t = rank_tile[s][k]
                    go = goff(s, t)
                    lhsT = aug[s][:, go:go + G_TILE]
                    o = j * Wg
                    done = 0
                    while done < Wg:
                        room = 512 - ((o + done) % 512)
                        w = min(room, Wg - done)
                        po = poff(s, k, int(offs[s][k]) + done)
                        mm = nc.tensor.matmul(
                            pt[i % 2][:, o + done:o + done + w],
                            lhsT,
                            aug[s][:, po:po + w],
                            start=True, stop=True,
                        )
                        done += w
                    if half and j == half - 1:
                        mm.then_inc(pe_sem, 1)
                mm.then_inc(pe_sem, 2 if not half else 1)

        @block.scalar
        def _(scalar):
            for i, (s, r0, nt, Wg) in enumerate(groups):
                half = nt // 2 if nt >= 4 else 0
                scalar.wait_ge(pe_sem, 2 * i + 1)
                if i >= NB:
                    scalar.wait_ge(out_sem, 32 * (i - NB + 1))
                if not half:
                    scalar.wait_ge(pe_sem, 2 * i + 2)
                    nc.scalar.activation(
                        d2s[:, i % NB, 0:nt * Wg],
                        pt[i % 2][:, 0:nt * Wg],
                        mybir.ActivationFunctionType.Copy, scale=D2_SCALE,
                    ).then_inc(act_sem, 2)
                    continue
                cut = half * Wg
                nc.scalar.activation(
                    d2s[:, i % NB, 0:cut],
                    pt[i % 2][:, 0:cut],
                    mybir.ActivationFunctionType.Copy, scale=D2_SCALE,
                ).then_inc(act_sem, 1)
                scalar.wait_ge(pe_sem, 2 * i + 2)
                nc.scalar.activation(
                    d2s[:, i % NB, cut:nt * Wg],
                    pt[i % 2][:, cut:nt * Wg],
                    mybir.ActivationFunctionType.Copy, scale=D2_SCALE,
                ).then_inc(act_sem, 1)

    return nc


def _loss_from_nn(d_g, d_p, n_g, n_p):
    with np.errstate(divide="ignore", invalid="ignore", over="ignore"):
        gth2pred = d_g.sum() / n_g if n_g > 0 else np.float64(np.nan)
        pred2gth = d_p.sum() / n_p if n_p > 0 else np.float64(np.nan)
        ahd = (gth2pred + pred2gth) / 2.0
        if n_g == 0 and n_p == 0:
            ahd = np.float64(np.nan)
        return 1.0 - 1.0 / (1.0 + ahd)


RUN_OPTS = {}
LAST_RES = None
LAST_INFO = {}


def kernel(gth, pred):
    from concourse.bass_utils import run_bass_kernel_spmd
    import ml_dtypes

    gth = np.asarray(gth, np.float32).reshape(BC, H, W_IMG)
    pred = np.asarray(pred, np.float32).reshape(BC, H, W_IMG)

    gedge = _edge_maps(gth)
    pedge = _edge_maps(pred)

    pts = []
    for i in range(BC):
        gy, gx = np.nonzero(gedge[i])
        py, px = np.nonzero(pedge[i])
        pts.append((gy.astype(np.int64), gx.astype(np.int64),
                    py.astype(np.int64), px.astype(np.int64)))

    n_gs = [len(p[0]) for p in pts]
    T = max(1, -(-max(n_gs) // G_TILE))
    pair_tiles, pair_reqs = [], []
    for i in range(BC):
        gy, gx, py, px = pts[i]
        n_g, n_p = len(gy), len(py)
        if n_g and n_p:
            u_g = _nn_upper_bound(_edt_full(pedge[i]), gy, gx)
            v_p = _nn_upper_bound(_edt_full(gedge[i]), py, px)
            tiles = _kd_tiles(gy, gx, T)
            reqs = _tile_reqs(tiles, gy, gx, py, px, u_g, v_p)
        else:
            tiles = [np.arange(min(n_g, G_TILE))] * T
            reqs = [np.arange(n_p)] * T
        pair_tiles.append(tiles)
        pair_reqs.append(reqs)

    pair_jobs = [_pair_jobs(pair_reqs[i]) for i in range(BC)]
    cost = [sum(_job_w(j) for j in jb) for jb in pair_jobs]
    order = sorted(range(BC), key=lambda i: -cost[i])
    assign = [[order[c], order[BC - 1 - c]] for c in range(N_CORES)]
    core_jobs = []
    for c in range(N_CORES):
        mj = ([(0,) + j for j in pair_jobs[assign[c][0]]]
              + [(1,) + j for j in pair_jobs[assign[c][1]]])
        mj.sort(key=lambda j: -len(j[2]))
        core_jobs.append(mj)
    w, o, perm, grp = _plan_slot(core_jobs)
    slot_w, slot_offs, slot_perm, slot_groups = [w], [o], [perm], [grp]

    # gaug tile layout: T quantile tiles + 1 sentinel tile per slot
    slot_T = [T + 1, T + 1]
    rank_tile = []
    for s in range(SLOTS):
        # rank k uses the tile of whichever pair; tile index must be common
        # across cores -> store per-rank tile as the job's tile for EACH core
        # in ITS OWN gaug. But lhsT slice index must be compile-time common!
        # Solution: gaug layout per core is REORDERED so that rank k's tile
        # data sits at gaug position k. ranks can exceed T (splits reuse the
        # same tile for several ranks; sentinel ranks use sentinel data).
        rank_tile.append(list(range(len(slot_w[s]))))
    slot_T = [len(slot_w[s]) for s in range(SLOTS)]
    _build_program.rank_tile = rank_tile

    nc = _build_program(slot_w, slot_T, slot_groups)

    in_maps = []
    core_maps = []   # per core: rank -> (pair01, tile, chunk) or None
    nrank = len(slot_w[0])
    C_s = int(slot_offs[0][-1])
    for c in range(N_CORES):
        jobs = core_jobs[c]
        cyg = np.full(nrank * G_TILE, SENT, np.float32)
        cxg = np.full(nrank * G_TILE, SENT, np.float32)
        cyp = np.full(C_s, SENT, np.float32)
        cxp = np.full(C_s, SENT, np.float32)
        rmap = []
        for k in range(nrank):
            jk = slot_perm[0][k]
            if jk >= len(jobs):
                rmap.append(None)
                continue
            p01, t, chunk = jobs[jk]
            i = assign[c][p01]
            gy, gx, py, px = pts[i]
            rows = pair_tiles[i][t]
            cyg[k * G_TILE:k * G_TILE + len(rows)] = gy[rows] - 128.0
            cxg[k * G_TILE:k * G_TILE + len(rows)] = gx[rows] - 128.0
            o = int(slot_offs[0][k])
            cyp[o:o + len(chunk)] = py[chunk] - 128.0
            cxp[o:o + len(chunk)] = px[chunk] - 128.0
            rmap.append((p01, t, chunk))
        ga = _aug_g(cyg, cxg)
        pa = _aug_p(cyp, cxp)
        n0h = slot_groups[0][0][1]
        g0wh = int(slot_offs[0][n0h])
        in_maps.append({"aug0": np.concatenate(
            [ga[:, :n0h * G_TILE], pa[:, :g0wh],
             ga[:, n0h * G_TILE:], pa[:, g0wh:]],
            axis=1).astype(ml_dtypes.bfloat16)})
        core_maps.append(rmap)

    res = run_bass_kernel_spmd(nc, in_maps, list(range(N_CORES)), **RUN_OPTS)
    global LAST_RES, LAST_INFO
    LAST_RES = res
    LAST_INFO = {"slot_w": slot_w, "assign": assign, "T": T}
    results = res.results

    losses = np.full(BC, np.nan, np.float64)
    for c in range(N_CORES):
        rmap = core_maps[c]
        dp_raw = np.asarray(results[c]["dp0"], np.float32)
        colmax = dp_raw.max(axis=0)
        val_g = [np.full((T, G_TILE), -np.inf, np.float32) for _ in range(2)]
        dpv = [np.full(max(len(pts[assign[c][p]][2]), 1), -np.inf, np.float32)
               for p in range(2)]
        for k in range(nrank):
            if rmap[k] is None:
                continue
            p01, t, chunk = rmap[k]
            Wk = slot_w[0][k]
            o = int(slot_offs[0][k])
            blk = dp_raw[:, o:o + Wk].max(axis=1)
            val_g[p01][t] = np.maximum(val_g[p01][t], blk)
            if len(chunk):
                np.maximum.at(dpv[p01], chunk, colmax[o:o + len(chunk)])
        for p01 in range(2):
            i = assign[c][p01]
            gy, gx, py, px = pts[i]
            n_g, n_p = len(gy), len(py)
            if n_g == 0 and n_p == 0:
                continue
            tiles = pair_tiles[i]
            dgv = np.empty(max(n_g, 1), np.float32)
            for t in range(T):
                rows = tiles[t]
                dgv[rows] = val_g[p01][t, :len(rows)]
            d_g = np.sqrt(np.maximum(
                D2_BACK * dgv[:n_g].astype(np.float64), 0.0))
            d_p = np.sqrt(np.maximum(
                D2_BACK * dpv[p01][:n_p].astype(np.float64), 0.0))
            losses[i] = _loss_from_nn(d_g, d_p, n_g, n_p)

    return np.float32(np.nanmean(losses.astype(np.float32)))



# revision 6
# speedup vs baseline: 1.0720x; 1.0720x over previous
"""Average Hausdorff loss on 8 Trainium2 NeuronCores — K-packed streamed KNN.

Host (numpy): edge detection, exact EDT for certified NN-distance upper
bounds, per-tile candidate sets (certificate + coverage), then a flat
per-core column stream cut into uniform 512-wide PSUM groups.  Within a
group, each column belongs to one (tile, chunk) segment; segment s of a
group occupies contract rows 6s..6s+5 of a zero-stuffed rhs, so ONE
matmul per group computes every tile's distances (lhsT stacks the
group's tiles along the contract dim).  This replaces the baseline's
per-tile matmul+LDWEIGHTS pairs (51 LDW / 51 MM, ~450ns each) with
NG=~11 large back-to-back matmuls.

Device (raw Bass, SPMD over 8 cores):
  PE : 5 warm-up dummy matmuls during the input-DMA dead time (ramps the
       HAM clock 1.2->2.4 GHz), then one 512-col matmul per group into a
       rotating PSUM bank -> PSUM = -(d^2)/4 exactly
  ACT: even groups PSUM->fp16 ring copy, then self-issued HWDGE DMA out
  DVE: odd groups PSUM->fp16 ring copy (sync engine issues their DMAs)
  DMA: fp16 512-col blocks stream to DRAM per group
Host: per-segment row maxes (gth->pred NN), column maxes scattered into
pred space (pred->gth NN), sqrt, means, nanmean.

Pad rows use a far sentinel coordinate (overflows to big-negative/-inf
in fp16 and always loses the max); pad columns are all-zero and are
never read back.
"""

import numpy as np

H = 256
W_IMG = 256
BC = 16
N_CORES = 8
G_TILE = 128
GW = 512          # group width (one PSUM bank)
NB = 7            # PSUM banks cycled by real groups (bank 7 = dummies)
ND_DUMMY = 5      # PE warm-up dummy matmuls
RING_S = 4        # fp16 ring slots for the scalar-copied groups
RING_V = 4        # fp16 ring slots for the vector-copied groups
SENTC = 512.0     # sentinel coordinate (centered); min d^2 to any real
                  # point is 2*385^2 = 296450 > max real d^2 130050
EDT_SLACK = 0.01


def _edge_maps(x):
    m = x > 0.5
    p = np.pad(m, ((0, 0), (1, 1), (1, 1)), constant_values=True)
    e = np.ones_like(m)
    for dy in range(3):
        for dx in range(3):
            e &= p[:, dy:dy + H, dx:dx + W_IMG]
    return m & ~e


def _edt_full(mask):
    """Exact EDT of `mask` ([256,256] bool) by two separable min passes."""
    BIG = np.float32(1e9)
    col = np.where(mask, np.float32(0.0), BIG)
    ar = np.arange(256, dtype=np.float32)
    d2 = (ar[:, None] - ar[None, :]) ** 2
    D1 = np.empty((256, 256), np.float32)
    D2 = np.empty((256, 256), np.float32)
    for c0 in range(0, 256, 64):
        D1[:, c0:c0 + 64] = (d2[:, :, None] + col[None, :, c0:c0 + 64]).min(1)
    for r0 in range(0, 256, 64):
        D2[r0:r0 + 64] = (D1[r0:r0 + 64, None, :] + d2[None, :, :]).min(2)
    return np.sqrt(D2)


def _nn_upper_bound(edt_other, ys, xs):
    return edt_other[ys, xs] + EDT_SLACK


def _aug_g(cy, cx):
    """6-row stationary augmentation (exact in bf16): dot with _aug_p
    gives -(d^2)/4."""
    n = cy.shape[0]
    out = np.zeros((6, n), np.float32)
    sq = cy * cy + cx * cx
    b1 = np.floor(sq / 256.0)
    b0 = sq - b1 * 256.0
    out[0] = cy * 0.5
    out[1] = cx * 0.5
    out[2] = -b1
    out[3] = -b0
    out[4] = -64.0
    out[5] = -0.25
    return out


def _aug_p(cy, cx):
    n = cy.shape[0]
    out = np.zeros((6, n), np.float32)
    sq = cy * cy + cx * cx
    b1 = np.floor(sq / 256.0)
    b0 = sq - b1 * 256.0
    out[0] = cy
    out[1] = cx
    out[2] = 64.0
    out[3] = 0.25
    out[4] = b1
    out[5] = b0
    return out


def _kd_tiles(gy, gx, T):
    """Split gth points into T spatially-local tiles of <=128 points
    (recursive median bisection, alternating axes)."""
    leaves = []

    def split(ids, nt, axis):
        if nt == 1:
            leaves.append(ids)
            return
        t1 = nt // 2
        keys = (gy[ids], gx[ids])[axis]
        order = np.argsort(keys, kind='stable')
        cut = (len(ids) * t1) // nt
        split(ids[order[:cut]], t1, 1 - axis)
        split(ids[order[cut:]], nt - t1, 1 - axis)

    split(np.arange(len(gy)), T, 0)
    return leaves


def _tile_reqs(tiles, gy, gx, py, px, u_g, v_p):
    """Per tile: array of pred indices that (a) could be the NN of a
    tile point (certificate disc) or (b) could have their NN in the tile
    (coverage disc)."""
    reqs = []
    for ids in tiles:
        ymin, ymax = gy[ids].min(), gy[ids].max()
        xmin, xmax = gx[ids].min(), gx[ids].max()
        U = u_g[ids].max()
        V = v_p.max() if len(v_p) else 0.0
        cand = np.nonzero(
            (py >= ymin - max(U, V)) & (py <= ymax + max(U, V))
            & (px >= xmin - max(U, V)) & (px <= xmax + max(U, V)))[0]
        if len(cand) == 0:
            reqs.append(cand)
            continue
        cy, cx, cv = py[cand], px[cand], v_p[cand]
        ty, tx, tu = gy[ids], gx[ids], u_g[ids]
        dd = ((cy[None, :] - ty[:, None]).astype(np.float32) ** 2
              + (cx[None, :] - tx[:, None]).astype(np.float32) ** 2)
        hit = (dd <= (tu[:, None] ** 2)).any(0)
        hit |= (dd <= (cv[None, :] ** 2)).any(0)
        reqs.append(cand[np.nonzero(hit)[0]])
    return reqs


def _loss_from_nn(d_g, d_p, n_g, n_p):
    with np.errstate(divide="ignore", invalid="ignore", over="ignore"):
        gth2pred = d_g.sum() / n_g if n_g > 0 else np.float64(np.nan)
        pred2gth = d_p.sum() / n_p if n_p > 0 else np.float64(np.nan)
        ahd = (gth2pred + pred2gth) / 2.0
        if n_g == 0 and n_p == 0:
            ahd = np.float64(np.nan)
        return 1.0 - 1.0 / (1.0 + ahd)


def _build_program(NG, PACK):
    """One 512-col matmul per group; groups cycle PSUM banks 0..NB-1.
    Copies alternate Scalar (even groups, self-issued out-DMA) and
    Vector (odd groups, out-DMA from the sync engine)."""
    from contextlib import ExitStack
    import concourse.bass as bass
    import concourse.mybir as mybir

    f32 = mybir.dt.float32
    f16 = mybir.dt.float16
    bf16 = mybir.dt.bfloat16
    K = 6 * PACK

    nc = bass.Bass()
    lhs_d = nc.declare_dram_parameter("lhs", [K, NG * G_TILE], bf16,
                                      isOutput=False)
    rhs_d = nc.declare_dram_parameter("rhs", [K, NG * GW], bf16,
                                      isOutput=False)
    dp_d = nc.declare_dram_parameter("dp0", [G_TILE, NG * GW], f16,
                                     isOutput=True)
    warm_d = nc.declare_dram_parameter("warm", [1, 16], f16, isOutput=True)

    evens = list(range(0, NG, 2))
    odds = list(range(1, NG, 2))
    # input chunks: (first group covered exclusively, sem threshold)
    # chunk 0: lhs + rhs groups [0,2); chunk 1: rhs [2,6); chunk 2: [6,NG)
    cut1, cut2 = min(2, NG), min(6, NG)

    def in_need(g):
        if g < cut1:
            return 32
        if g < cut2:
            return 48
        return 64

    with ExitStack() as ctx:
        lhs_s = ctx.enter_context(
            nc.sbuf_tensor("lhs_s", [K, NG * G_TILE], bf16))
        rhs_s = ctx.enter_context(
            nc.sbuf_tensor("rhs_s", [K, NG * GW], bf16))
        ring = ctx.enter_context(
            nc.sbuf_tensor("ring", [G_TILE, (RING_S + RING_V) * GW], f16))
        pt = ctx.enter_context(nc.psum_tensor("pt", [G_TILE, 4096], f32))

        in_sem = ctx.enter_context(nc.semaphore("in_sem"))
        pe_sem = ctx.enter_context(nc.semaphore("pe_sem"))
        sc_sem = ctx.enter_context(nc.semaphore("sc_sem"))
        vc_sem = ctx.enter_context(nc.semaphore("vc_sem"))
        sd_sem = ctx.enter_context(nc.semaphore("sd_sem"))
        vd_sem = ctx.enter_context(nc.semaphore("vd_sem"))
        wm_sem = ctx.enter_context(nc.semaphore("wm_sem"))
        block = ctx.enter_context(nc.Block())

        @block.sync
        def _(sync):
            # prime the SP HWDGE ring with a tiny dummy transfer
            sync.dma_start(warm_d[0:1, 0:8], ring[0:1, 0:8]
                           ).then_inc(wm_sem, 16)
            # input chunks (each DMA incs in_sem by 16)
            sync.dma_start(lhs_s[:, :], lhs_d[:, :]).then_inc(in_sem, 16)
            sync.dma_start(rhs_s[:, 0:cut1 * GW],
                           rhs_d[:, 0:cut1 * GW]).then_inc(in_sem, 16)
            sync.dma_start(rhs_s[:, cut1 * GW:cut2 * GW],
                           rhs_d[:, cut1 * GW:cut2 * GW]).then_inc(in_sem, 16)
            sync.dma_start(rhs_s[:, cut2 * GW:],
                           rhs_d[:, cut2 * GW:]).then_inc(in_sem, 16)
            # vector-half output DMAs
            for k, g in enumerate(odds):
                sync.wait_ge(vc_sem, k + 1)
                slot = RING_S + (k % RING_V)
                sync.dma_start(dp_d[:, g * GW:(g + 1) * GW],
                               ring[:, slot * GW:(slot + 1) * GW],
                               ).then_inc(vd_sem, 16)

        @block.tensor
        def _(tensor):
            # HAM warm-up: dummy matmuls on stale SBUF into PSUM bank 7
            for _i in range(ND_DUMMY):
                nc.tensor.matmul(pt[:, NB * GW:(NB + 1) * GW],
                                 rhs_s[:, 0:G_TILE], rhs_s[:, 0:GW],
                                 start=True, stop=True)
            cur_need = 0
            for g in range(NG):
                need = in_need(g)
                if need > cur_need:
                    tensor.wait_ge(in_sem, need)
                    cur_need = need
                if g >= NB:
                    gp = g - NB
                    if gp % 2 == 0:
                        tensor.wait_ge(sc_sem, gp // 2 + 1)
                    else:
                        tensor.wait_ge(vc_sem, gp // 2 + 1)
                b = g % NB
                nc.tensor.matmul(
                    pt[:, b * GW:(b + 1) * GW],
                    lhs_s[:, g * G_TILE:(g + 1) * G_TILE],
                    rhs_s[:, g * GW:(g + 1) * GW],
                    start=True, stop=True,
                ).then_inc(pe_sem, 1)

        @block.scalar
        def _(scalar):
            # load the activation table + prime the ACT HWDGE ring early
            nc.scalar.activation(ring[0:1, 0:8], ring[0:1, 8:16],
                                 mybir.ActivationFunctionType.Copy, scale=1.0)
            nc.scalar.dma_start(warm_d[0:1, 8:16], ring[0:1, 0:8]
                                ).then_inc(wm_sem, 16)
            for k, g in enumerate(evens):
                scalar.wait_ge(pe_sem, g + 1)
                if k >= RING_S:
                    scalar.wait_ge(sd_sem, 16 * (k - RING_S + 1))
                slot = k % RING_S
                b = g % NB
                nc.scalar.activation(
                    ring[:, slot * GW:(slot + 1) * GW],
                    pt[:, b * GW:(b + 1) * GW],
                    mybir.ActivationFunctionType.Copy, scale=1.0,
                ).then_inc(sc_sem, 1)
                nc.scalar.dma_start(dp_d[:, g * GW:(g + 1) * GW],
                                    ring[:, slot * GW:(slot + 1) * GW],
                                    ).then_inc(sd_sem, 16)

        @block.vector
        def _(vector):
            for k, g in enumerate(odds):
                vector.wait_ge(pe_sem, g + 1)
                if k >= RING_V:
                    vector.wait_ge(vd_sem, 16 * (k - RING_V + 1))
                slot = RING_S + (k % RING_V)
                b = g % NB
                nc.vector.tensor_copy(
                    ring[:, slot * GW:(slot + 1) * GW],
                    pt[:, b * GW:(b + 1) * GW],
                ).then_inc(vc_sem, 1)

    return nc


RUN_OPTS = {}
LAST_RES = None
LAST_INFO = {}


def kernel(gth, pred):
    from concourse.bass_utils import run_bass_kernel_spmd
    import ml_dtypes

    gth = np.asarray(gth, np.float32).reshape(BC, H, W_IMG)
    pred = np.asarray(pred, np.float32).reshape(BC, H, W_IMG)

    gedge = _edge_maps(gth)
    pedge = _edge_maps(pred)

    pts = []
    for i in range(BC):
        gy, gx = np.nonzero(gedge[i])
        py, px = np.nonzero(pedge[i])
        pts.append((gy.astype(np.int64), gx.astype(np.int64),
                    py.astype(np.int64), px.astype(np.int64)))

    pair_tiles, pair_reqs = [], []
    for i in range(BC):
        gy, gx, py, px = pts[i]
        n_g, n_p = len(gy), len(py)
        if n_g and n_p:
            u_g = _nn_upper_bound(_edt_full(pedge[i]), gy, gx)
            v_p = _nn_upper_bound(_edt_full(gedge[i]), py, px)
            T_i = max(1, -(-n_g // G_TILE))
            tiles = _kd_tiles(gy, gx, T_i)
            reqs = _tile_reqs(tiles, gy, gx, py, px, u_g, v_p)
        else:
            tiles, reqs = [], []
        pair_tiles.append(tiles)
        pair_reqs.append(reqs)

    raw = [sum(len(r) for r in pair_reqs[i]) for i in range(BC)]
    order = sorted(range(BC), key=lambda i: -raw[i])
    assign = [[order[c], order[BC - 1 - c]] for c in range(N_CORES)]

    # Per core: flat column stream of (pair01, tile, cand-slice) cut at
    # 512-col group boundaries.
    core_groups = []   # per core: per group: list of (p01,t,cand,ofs)
    for c in range(N_CORES):
        groups, cur, used = [], [], 0
        for p01 in (0, 1):
            i = assign[c][p01]
            for t, r in enumerate(pair_reqs[i]):
                pos = 0
                while pos < len(r):
                    take = min(GW - used, len(r) - pos)
                    cur.append((p01, t, r[pos:pos + take], used))
                    used += take
                    pos += take
                    if used == GW:
                        groups.append(cur)
                        cur, used = [], 0
        if cur:
            groups.append(cur)
        core_groups.append(groups)

    NG = max(1, max(len(g) for g in core_groups))
    PACK = max(2, max((len(seglist) for groups in core_groups
                       for seglist in groups), default=2))
    K = 6 * PACK

    nc = _build_program(NG, PACK)

    in_maps = []
    for c in range(N_CORES):
        lhs = np.zeros((K, NG * G_TILE), np.float32)
        rhs = np.zeros((K, NG * GW), np.float32)
        for g, seglist in enumerate(core_groups[c]):
            for s, (p01, t, cand, ofs) in enumerate(seglist):
                i = assign[c][p01]
                gy, gx, py, px = pts[i]
                rows = pair_tiles[i][t]
                cyg = np.full(G_TILE, SENTC, np.float32)
                cxg = np.full(G_TILE, SENTC, np.float32)
                cyg[:len(rows)] = gy[rows] - 128.0
                cxg[:len(rows)] = gx[rows] - 128.0
                lhs[6 * s:6 * s + 6, g * G_TILE:(g + 1) * G_TILE] = \
                    _aug_g(cyg, cxg)
                rhs[6 * s:6 * s + 6,
                    g * GW + ofs:g * GW + ofs + len(cand)] = \
                    _aug_p(py[cand] - 128.0, px[cand] - 128.0)
        in_maps.append({
            "lhs": lhs.astype(ml_dtypes.bfloat16),
            "rhs": rhs.astype(ml_dtypes.bfloat16),
        })

    res = run_bass_kernel_spmd(nc, in_maps, list(range(N_CORES)), **RUN_OPTS)
    global LAST_RES, LAST_INFO
    LAST_RES = res
    LAST_INFO = {"NG": NG, "PACK": PACK, "assign": assign}
    results = res.results

    losses = np.full(BC, np.nan, np.float64)
    for c in range(N_CORES):
        dp_raw = np.asarray(results[c]["dp0"], np.float32)
        colmax = dp_raw.max(axis=0)
        val_g = [None, None]
        dpv = [None, None]
        for p01 in (0, 1):
            i = assign[c][p01]
            nt = len(pair_tiles[i])
            val_g[p01] = np.full((max(nt, 1), G_TILE), -np.inf, np.float32)
            dpv[p01] = np.full(max(len(pts[i][2]), 1), -np.inf, np.float32)
        for g, seglist in enumerate(core_groups[c]):
            for (p01, t, cand, ofs) in seglist:
                c0 = g * GW + ofs
                blk = dp_raw[:, c0:c0 + len(cand)].max(axis=1)
                val_g[p01][t] = np.maximum(val_g[p01][t], blk)
                np.maximum.at(dpv[p01], cand, colmax[c0:c0 + len(cand)])
        for p01 in (0, 1):
            i = assign[c][p01]
            gy, gx, py, px = pts[i]
            n_g, n_p = len(gy), len(py)
            if n_g == 0 or n_p == 0:
                # reference yields nan whenever either set is empty
                losses[i] = np.nan
                continue
            tiles = pair_tiles[i]
            dgv = np.empty(n_g, np.float32)
            for t in range(len(tiles)):
                rows = tiles[t]
                dgv[rows] = val_g[p01][t, :len(rows)]
            d_g = np.sqrt(np.maximum(-4.0 * dgv.astype(np.float64), 0.0))
            d_p = np.sqrt(np.maximum(
                -4.0 * dpv[p01][:n_p].astype(np.float64), 0.0))
            losses[i] = _loss_from_nn(d_g, d_p, n_g, n_p)

    return np.float32(np.nanmean(losses.astype(np.float32)))


# revision 10
# speedup vs baseline: 1.1168x; 1.0418x over previous
"""Average Hausdorff loss on 8 Trainium2 NeuronCores — K-packed streamed KNN.

Host (numpy): edge detection, exact EDT for certified NN-distance upper
bounds, per-tile candidate sets (certificate + coverage), then a flat
per-core column stream cut into uniform 512-wide PSUM groups.  Within a
group, each column belongs to one (tile, chunk) segment; segment s of a
group occupies contract rows 6s..6s+5 of a zero-stuffed rhs, so ONE
matmul per group computes every tile's distances (lhsT stacks the
group's tiles along the contract dim).  This replaces the baseline's
per-tile matmul+LDWEIGHTS pairs (51 LDW / 51 MM, ~450ns each) with
NG=~11 large back-to-back matmuls.

Device (raw Bass, SPMD over 8 cores):
  PE : 5 warm-up dummy matmuls during the input-DMA dead time (ramps the
       HAM clock 1.2->2.4 GHz), then one 512-col matmul per group into a
       rotating PSUM bank -> PSUM = -(d^2)/4 exactly
  ACT: even groups PSUM->fp16 ring copy, then self-issued HWDGE DMA out
  DVE: odd groups PSUM->fp16 ring copy (sync engine issues their DMAs)
  DMA: fp16 512-col blocks stream to DRAM per group
Host: per-segment row maxes (gth->pred NN), column maxes scattered into
pred space (pred->gth NN), sqrt, means, nanmean.

Pad rows use a far sentinel coordinate (overflows to big-negative/-inf
in fp16 and always loses the max); pad columns are all-zero and are
never read back.
"""

import numpy as np

H = 256
W_IMG = 256
BC = 16
N_CORES = 8
G_TILE = 128
GW = 512          # group width (one PSUM bank)
NB = 7            # PSUM banks cycled by real groups (bank 7 = dummies)
ND_DUMMY = 8      # PE warm-up dummy matmuls
RING_S = 4        # fp16 ring slots for the scalar-copied groups
RING_V = 4        # fp16 ring slots for the vector-copied groups
SENTC = 512.0     # sentinel coordinate (centered); min d^2 to any real
                  # point is 2*385^2 = 296450 > max real d^2 130050
EDT_SLACK = 0.01


def _edge_maps(x):
    m = x > 0.5
    p = np.pad(m, ((0, 0), (1, 1), (1, 1)), constant_values=True)
    e = np.ones_like(m)
    for dy in range(3):
        for dx in range(3):
            e &= p[:, dy:dy + H, dx:dx + W_IMG]
    return m & ~e


def _edt_full(mask):
    """Exact EDT of `mask` ([256,256] bool) by two separable min passes."""
    BIG = np.float32(1e9)
    col = np.where(mask, np.float32(0.0), BIG)
    ar = np.arange(256, dtype=np.float32)
    d2 = (ar[:, None] - ar[None, :]) ** 2
    D1 = np.empty((256, 256), np.float32)
    D2 = np.empty((256, 256), np.float32)
    for c0 in range(0, 256, 64):
        D1[:, c0:c0 + 64] = (d2[:, :, None] + col[None, :, c0:c0 + 64]).min(1)
    for r0 in range(0, 256, 64):
        D2[r0:r0 + 64] = (D1[r0:r0 + 64, None, :] + d2[None, :, :]).min(2)
    return np.sqrt(D2)


def _nn_upper_bound(edt_other, ys, xs):
    return edt_other[ys, xs] + EDT_SLACK


def _aug_g(cy, cx):
    """6-row stationary augmentation (exact in bf16): dot with _aug_p
    gives -(d^2)/4."""
    n = cy.shape[0]
    out = np.zeros((6, n), np.float32)
    sq = cy * cy + cx * cx
    b1 = np.floor(sq / 256.0)
    b0 = sq - b1 * 256.0
    out[0] = cy * 0.5
    out[1] = cx * 0.5
    out[2] = -b1
    out[3] = -b0
    out[4] = -64.0
    out[5] = -0.25
    return out


def _aug_p(cy, cx):
    n = cy.shape[0]
    out = np.zeros((6, n), np.float32)
    sq = cy * cy + cx * cx
    b1 = np.floor(sq / 256.0)
    b0 = sq - b1 * 256.0
    out[0] = cy
    out[1] = cx
    out[2] = 64.0
    out[3] = 0.25
    out[4] = b1
    out[5] = b0
    return out


def _kd_tiles(gy, gx, T):
    """Split gth points into T spatially-local tiles of <=128 points
    (recursive median bisection, alternating axes)."""
    leaves = []

    def split(ids, nt, axis):
        if nt == 1:
            leaves.append(ids)
            return
        t1 = nt // 2
        keys = (gy[ids], gx[ids])[axis]
        order = np.argsort(keys, kind='stable')
        cut = (len(ids) * t1) // nt
        split(ids[order[:cut]], t1, 1 - axis)
        split(ids[order[cut:]], nt - t1, 1 - axis)

    split(np.arange(len(gy)), T, 0)
    return leaves


def _tile_reqs(tiles, gy, gx, py, px, u_g, v_p):
    """Per tile: array of pred indices that (a) could be the NN of a
    tile point (certificate disc) or (b) could have their NN in the tile
    (coverage disc)."""
    reqs = []
    for ids in tiles:
        ymin, ymax = gy[ids].min(), gy[ids].max()
        xmin, xmax = gx[ids].min(), gx[ids].max()
        U = u_g[ids].max()
        V = v_p.max() if len(v_p) else 0.0
        cand = np.nonzero(
            (py >= ymin - max(U, V)) & (py <= ymax + max(U, V))
            & (px >= xmin - max(U, V)) & (px <= xmax + max(U, V)))[0]
        if len(cand) == 0:
            reqs.append(cand)
            continue
        cy, cx, cv = py[cand], px[cand], v_p[cand]
        ty, tx, tu = gy[ids], gx[ids], u_g[ids]
        dd = ((cy[None, :] - ty[:, None]).astype(np.float32) ** 2
              + (cx[None, :] - tx[:, None]).astype(np.float32) ** 2)
        hit = (dd <= (tu[:, None] ** 2)).any(0)
        hit |= (dd <= (cv[None, :] ** 2)).any(0)
        reqs.append(cand[np.nonzero(hit)[0]])
    return reqs


def _loss_from_nn(d_g, d_p, n_g, n_p):
    with np.errstate(divide="ignore", invalid="ignore", over="ignore"):
        gth2pred = d_g.sum() / n_g if n_g > 0 else np.float64(np.nan)
        pred2gth = d_p.sum() / n_p if n_p > 0 else np.float64(np.nan)
        ahd = (gth2pred + pred2gth) / 2.0
        if n_g == 0 and n_p == 0:
            ahd = np.float64(np.nan)
        return 1.0 - 1.0 / (1.0 + ahd)


def _build_program(NG, PACK):
    """One 512-col matmul per group; group g accumulates into PSUM bank
    g%8 (dummy warm-up matmuls use bank 7, overwritten by group 7).
    Copies run in 1024-col units alternating Scalar/Vector; all output
    DMAs are issued by the sync engine (per copy unit)."""
    from contextlib import ExitStack
    import concourse.bass as bass
    import concourse.mybir as mybir

    f32 = mybir.dt.float32
    f16 = mybir.dt.float16
    bf16 = mybir.dt.bfloat16
    K = 6 * PACK
    LOFS = NG * G_TILE          # rhs column offset inside the packed input

    nc = bass.Bass()
    inp_d = nc.declare_dram_parameter("inp", [K, LOFS + NG * GW], bf16,
                                      isOutput=False)
    dp_d = nc.declare_dram_parameter("dp0", [G_TILE, NG * GW], f16,
                                     isOutput=True)

    # copy/DMA units: pairs of groups (last unit may be a single group)
    units = [(g, min(g + 1, NG - 1)) for g in range(0, NG, 2)]

    # input chunks: chunk0 = lhs + rhs groups [0,2); chunk1 = [2,6);
    # chunk2 = [6,NG)
    cut1, cut2 = min(2, NG), min(6, NG)

    def in_need(g):
        if g < cut1:
            return 16
        if g < cut2:
            return 32
        return 48

    with ExitStack() as ctx:
        inp_s = ctx.enter_context(
            nc.sbuf_tensor("inp_s", [K, LOFS + NG * GW], bf16))
        ring = ctx.enter_context(
            nc.sbuf_tensor("ring", [G_TILE, NG * GW], f16))
        pt = ctx.enter_context(nc.psum_tensor("pt", [G_TILE, 4096], f32))

        in_sem = ctx.enter_context(nc.semaphore("in_sem"))
        pe_sem = ctx.enter_context(nc.semaphore("pe_sem"))
        sc_sem = ctx.enter_context(nc.semaphore("sc_sem"))
        vc_sem = ctx.enter_context(nc.semaphore("vc_sem"))
        od_sem = ctx.enter_context(nc.semaphore("od_sem"))
        block = ctx.enter_context(nc.Block())

        def wait_unit(eng, u):
            if u % 2 == 0:
                eng.wait_ge(sc_sem, u // 2 + 1)
            else:
                eng.wait_ge(vc_sem, (u - 1) // 2 + 1)

        @block.sync
        def _(sync):
            sync.dma_start(inp_s[:, 0:LOFS + cut1 * GW],
                           inp_d[:, 0:LOFS + cut1 * GW]).then_inc(in_sem, 16)
            sync.dma_start(inp_s[:, LOFS + cut1 * GW:LOFS + cut2 * GW],
                           inp_d[:, LOFS + cut1 * GW:LOFS + cut2 * GW],
                           ).then_inc(in_sem, 16)
            sync.dma_start(inp_s[:, LOFS + cut2 * GW:],
                           inp_d[:, LOFS + cut2 * GW:]).then_inc(in_sem, 16)
            for u, (g0, g1) in enumerate(units):
                wait_unit(sync, u)
                sync.dma_start(dp_d[:, g0 * GW:(g1 + 1) * GW],
                               ring[:, g0 * GW:(g1 + 1) * GW],
                               ).then_inc(od_sem, 16)

        @block.tensor
        def _(tensor):
            # HAM warm-up: dummy matmuls on stale SBUF into PSUM bank 7
            for _i in range(ND_DUMMY):
                nc.tensor.matmul(pt[:, 7 * GW:8 * GW],
                                 inp_s[:, 0:G_TILE], inp_s[:, 0:GW],
                                 start=True, stop=True)
            cur_need = 0
            for g in range(NG):
                need = in_need(g)
                if need > cur_need:
                    tensor.wait_ge(in_sem, need)
                    cur_need = need
                if g >= 8:
                    wait_unit(tensor, (g - 8) // 2)
                b = g % 8
                nc.tensor.matmul(
                    pt[:, b * GW:(b + 1) * GW],
                    inp_s[:, g * G_TILE:(g + 1) * G_TILE],
                    inp_s[:, LOFS + g * GW:LOFS + (g + 1) * GW],
                    start=True, stop=True,
                ).then_inc(pe_sem, 1)

        @block.scalar
        def _(scalar):
            # trigger the activation-table load during the input dead time
            nc.scalar.activation(ring[0:1, 0:8], ring[0:1, 8:16],
                                 mybir.ActivationFunctionType.Copy, scale=1.0)
            for u, (g0, g1) in enumerate(units):
                if u % 2 != 0:
                    continue
                scalar.wait_ge(pe_sem, g1 + 1)
                nc.scalar.activation(
                    ring[:, g0 * GW:(g1 + 1) * GW],
                    pt[:, (g0 % 8) * GW:(g1 % 8 + 1) * GW],
                    mybir.ActivationFunctionType.Copy, scale=1.0,
                ).then_inc(sc_sem, 1)

        @block.vector
        def _(vector):
            for u, (g0, g1) in enumerate(units):
                if u % 2 != 1:
                    continue
                vector.wait_ge(pe_sem, g1 + 1)
                nc.vector.tensor_copy(
                    ring[:, g0 * GW:(g1 + 1) * GW],
                    pt[:, (g0 % 8) * GW:(g1 % 8 + 1) * GW],
                ).then_inc(vc_sem, 1)

    return nc


RUN_OPTS = {}
LAST_RES = None
LAST_INFO = {}


def kernel(gth, pred):
    from concourse.bass_utils import run_bass_kernel_spmd
    import ml_dtypes

    gth = np.asarray(gth, np.float32).reshape(BC, H, W_IMG)
    pred = np.asarray(pred, np.float32).reshape(BC, H, W_IMG)

    gedge = _edge_maps(gth)
    pedge = _edge_maps(pred)

    pts = []
    for i in range(BC):
        gy, gx = np.nonzero(gedge[i])
        py, px = np.nonzero(pedge[i])
        pts.append((gy.astype(np.int64), gx.astype(np.int64),
                    py.astype(np.int64), px.astype(np.int64)))

    pair_tiles, pair_reqs = [], []
    for i in range(BC):
        gy, gx, py, px = pts[i]
        n_g, n_p = len(gy), len(py)
        if n_g and n_p:
            u_g = _nn_upper_bound(_edt_full(pedge[i]), gy, gx)
            v_p = _nn_upper_bound(_edt_full(gedge[i]), py, px)
            T_i = max(1, -(-n_g // G_TILE))
            tiles = _kd_tiles(gy, gx, T_i)
            reqs = _tile_reqs(tiles, gy, gx, py, px, u_g, v_p)
        else:
            tiles, reqs = [], []
        pair_tiles.append(tiles)
        pair_reqs.append(reqs)

    raw = [sum(len(r) for r in pair_reqs[i]) for i in range(BC)]
    order = sorted(range(BC), key=lambda i: -raw[i])
    assign = [[order[c], order[BC - 1 - c]] for c in range(N_CORES)]

    # Per core: flat column stream of (pair01, tile, cand-slice) cut at
    # 512-col group boundaries.
    core_groups = []   # per core: per group: list of (p01,t,cand,ofs)
    for c in range(N_CORES):
        groups, cur, used = [], [], 0
        for p01 in (0, 1):
            i = assign[c][p01]
            for t, r in enumerate(pair_reqs[i]):
                pos = 0
                while pos < len(r):
                    take = min(GW - used, len(r) - pos)
                    cur.append((p01, t, r[pos:pos + take], used))
                    used += take
                    pos += take
                    if used == GW:
                        groups.append(cur)
                        cur, used = [], 0
        if cur:
            groups.append(cur)
        core_groups.append(groups)

    NG = max(1, max(len(g) for g in core_groups))
    PACK = max(2, max((len(seglist) for groups in core_groups
                       for seglist in groups), default=2))
    K = 6 * PACK

    nc = _build_program(NG, PACK)

    LOFS = NG * G_TILE
    in_maps = []
    for c in range(N_CORES):
        inp = np.zeros((K, LOFS + NG * GW), np.float32)
        for g, seglist in enumerate(core_groups[c]):
            for s, (p01, t, cand, ofs) in enumerate(seglist):
                i = assign[c][p01]
                gy, gx, py, px = pts[i]
                rows = pair_tiles[i][t]
                cyg = np.full(G_TILE, SENTC, np.float32)
                cxg = np.full(G_TILE, SENTC, np.float32)
                cyg[:len(rows)] = gy[rows] - 128.0
                cxg[:len(rows)] = gx[rows] - 128.0
                inp[6 * s:6 * s + 6, g * G_TILE:(g + 1) * G_TILE] = \
                    _aug_g(cyg, cxg)
                inp[6 * s:6 * s + 6,
                    LOFS + g * GW + ofs:LOFS + g * GW + ofs + len(cand)] = \
                    _aug_p(py[cand] - 128.0, px[cand] - 128.0)
        in_maps.append({"inp": inp.astype(ml_dtypes.bfloat16)})

    res = run_bass_kernel_spmd(nc, in_maps, list(range(N_CORES)), **RUN_OPTS)
    global LAST_RES, LAST_INFO
    LAST_RES = res
    LAST_INFO = {"NG": NG, "PACK": PACK, "assign": assign}
    results = res.results

    losses = np.full(BC, np.nan, np.float64)
    for c in range(N_CORES):
        dp_raw = np.asarray(results[c]["dp0"], np.float32)
        colmax = dp_raw.max(axis=0)
        val_g = [None, None]
        dpv = [None, None]
        for p01 in (0, 1):
            i = assign[c][p01]
            nt = len(pair_tiles[i])
            val_g[p01] = np.full((max(nt, 1), G_TILE), -np.inf, np.float32)
            dpv[p01] = np.full(max(len(pts[i][2]), 1), -np.inf, np.float32)
        for g, seglist in enumerate(core_groups[c]):
            for (p01, t, cand, ofs) in seglist:
                c0 = g * GW + ofs
                blk = dp_raw[:, c0:c0 + len(cand)].max(axis=1)
                val_g[p01][t] = np.maximum(val_g[p01][t], blk)
                np.maximum.at(dpv[p01], cand, colmax[c0:c0 + len(cand)])
        for p01 in (0, 1):
            i = assign[c][p01]
            gy, gx, py, px = pts[i]
            n_g, n_p = len(gy), len(py)
            if n_g == 0 or n_p == 0:
                # reference yields nan whenever either set is empty
                losses[i] = np.nan
                continue
            tiles = pair_tiles[i]
            dgv = np.empty(n_g, np.float32)
            for t in range(len(tiles)):
                rows = tiles[t]
                dgv[rows] = val_g[p01][t, :len(rows)]
            d_g = np.sqrt(np.maximum(-4.0 * dgv.astype(np.float64), 0.0))
            d_p = np.sqrt(np.maximum(
                -4.0 * dpv[p01][:n_p].astype(np.float64), 0.0))
            losses[i] = _loss_from_nn(d_g, d_p, n_g, n_p)

    return np.float32(np.nanmean(losses.astype(np.float32)))


# revision 12
# speedup vs baseline: 1.1447x; 1.0250x over previous
"""Average Hausdorff loss on 8 Trainium2 NeuronCores — K-packed streamed KNN.

Host (numpy): edge detection, exact EDT for certified NN-distance upper
bounds, per-tile candidate sets (certificate + coverage), then a flat
per-core column stream cut into uniform 512-wide PSUM groups.  Within a
group, each column belongs to one (tile, chunk) segment; segment s of a
group occupies contract rows 6s..6s+5 of a zero-stuffed rhs, so ONE
matmul per group computes every tile's distances (lhsT stacks the
group's tiles along the contract dim).  This replaces the baseline's
per-tile matmul+LDWEIGHTS pairs (51 LDW / 51 MM, ~450ns each) with
NG=~11 large back-to-back matmuls.

Device (raw Bass, SPMD over 8 cores):
  PE : 5 warm-up dummy matmuls during the input-DMA dead time (ramps the
       HAM clock 1.2->2.4 GHz), then one 512-col matmul per group into a
       rotating PSUM bank -> PSUM = -(d^2)/4 exactly
  ACT: even groups PSUM->fp16 ring copy, then self-issued HWDGE DMA out
  DVE: odd groups PSUM->fp16 ring copy (sync engine issues their DMAs)
  DMA: fp16 512-col blocks stream to DRAM per group
Host: per-segment row maxes (gth->pred NN), column maxes scattered into
pred space (pred->gth NN), sqrt, means, nanmean.

Pad rows use a far sentinel coordinate (overflows to big-negative/-inf
in fp16 and always loses the max); pad columns are all-zero and are
never read back.
"""

import numpy as np

H = 256
W_IMG = 256
BC = 16
N_CORES = 8
G_TILE = 128
GW = 512          # group width (one PSUM bank)
NB = 7            # PSUM banks cycled by real groups (bank 7 = dummies)
ND_DUMMY = 4      # PE warm-up dummy matmuls
RING_S = 4        # fp16 ring slots for the scalar-copied groups
RING_V = 4        # fp16 ring slots for the vector-copied groups
SENTC = 512.0     # sentinel coordinate (centered); min d^2 to any real
                  # point is 2*385^2 = 296450 > max real d^2 130050
EDT_SLACK = 0.01


def _edge_maps(x):
    m = x > 0.5
    p = np.pad(m, ((0, 0), (1, 1), (1, 1)), constant_values=True)
    e = np.ones_like(m)
    for dy in range(3):
        for dx in range(3):
            e &= p[:, dy:dy + H, dx:dx + W_IMG]
    return m & ~e


def _edt_full(mask):
    """Exact EDT of `mask` ([256,256] bool) by two separable min passes."""
    BIG = np.float32(1e9)
    col = np.where(mask, np.float32(0.0), BIG)
    ar = np.arange(256, dtype=np.float32)
    d2 = (ar[:, None] - ar[None, :]) ** 2
    D1 = np.empty((256, 256), np.float32)
    D2 = np.empty((256, 256), np.float32)
    for c0 in range(0, 256, 64):
        D1[:, c0:c0 + 64] = (d2[:, :, None] + col[None, :, c0:c0 + 64]).min(1)
    for r0 in range(0, 256, 64):
        D2[r0:r0 + 64] = (D1[r0:r0 + 64, None, :] + d2[None, :, :]).min(2)
    return np.sqrt(D2)


def _nn_upper_bound(edt_other, ys, xs):
    return edt_other[ys, xs] + EDT_SLACK


def _aug_g(cy, cx):
    """6-row stationary augmentation (exact in bf16): dot with _aug_p
    gives -(d^2)/4."""
    n = cy.shape[0]
    out = np.zeros((6, n), np.float32)
    sq = cy * cy + cx * cx
    b1 = np.floor(sq / 256.0)
    b0 = sq - b1 * 256.0
    out[0] = cy * 0.5
    out[1] = cx * 0.5
    out[2] = -b1
    out[3] = -b0
    out[4] = -64.0
    out[5] = -0.25
    return out


def _aug_p(cy, cx):
    n = cy.shape[0]
    out = np.zeros((6, n), np.float32)
    sq = cy * cy + cx * cx
    b1 = np.floor(sq / 256.0)
    b0 = sq - b1 * 256.0
    out[0] = cy
    out[1] = cx
    out[2] = 64.0
    out[3] = 0.25
    out[4] = b1
    out[5] = b0
    return out


def _kd_tiles(gy, gx, T):
    """Split gth points into T spatially-local tiles of <=128 points
    (recursive median bisection, alternating axes)."""
    leaves = []

    def split(ids, nt, axis):
        if nt == 1:
            leaves.append(ids)
            return
        t1 = nt // 2
        keys = (gy[ids], gx[ids])[axis]
        order = np.argsort(keys, kind='stable')
        cut = (len(ids) * t1) // nt
        split(ids[order[:cut]], t1, 1 - axis)
        split(ids[order[cut:]], nt - t1, 1 - axis)

    split(np.arange(len(gy)), T, 0)
    return leaves


def _tile_reqs(tiles, gy, gx, py, px, u_g, v_p):
    """Per tile: array of pred indices that (a) could be the NN of a
    tile point (certificate disc) or (b) could have their NN in the tile
    (coverage disc)."""
    reqs = []
    for ids in tiles:
        ymin, ymax = gy[ids].min(), gy[ids].max()
        xmin, xmax = gx[ids].min(), gx[ids].max()
        U = u_g[ids].max()
        V = v_p.max() if len(v_p) else 0.0
        cand = np.nonzero(
            (py >= ymin - max(U, V)) & (py <= ymax + max(U, V))
            & (px >= xmin - max(U, V)) & (px <= xmax + max(U, V)))[0]
        if len(cand) == 0:
            reqs.append(cand)
            continue
        cy, cx, cv = py[cand], px[cand], v_p[cand]
        ty, tx, tu = gy[ids], gx[ids], u_g[ids]
        dd = ((cy[None, :] - ty[:, None]).astype(np.float32) ** 2
              + (cx[None, :] - tx[:, None]).astype(np.float32) ** 2)
        hit = (dd <= (tu[:, None] ** 2)).any(0)
        hit |= (dd <= (cv[None, :] ** 2)).any(0)
        reqs.append(cand[np.nonzero(hit)[0]])
    return reqs


def _loss_from_nn(d_g, d_p, n_g, n_p):
    with np.errstate(divide="ignore", invalid="ignore", over="ignore"):
        gth2pred = d_g.sum() / n_g if n_g > 0 else np.float64(np.nan)
        pred2gth = d_p.sum() / n_p if n_p > 0 else np.float64(np.nan)
        ahd = (gth2pred + pred2gth) / 2.0
        if n_g == 0 and n_p == 0:
            ahd = np.float64(np.nan)
        return 1.0 - 1.0 / (1.0 + ahd)


def _build_program(NG, PACK):
    """One 512-col matmul per group; group g accumulates into PSUM bank
    g%8 (dummy warm-up matmuls use bank 7, overwritten by group 7).
    Copies run in 1024-col units alternating Scalar/Vector; all output
    DMAs are issued by the sync engine (per copy unit)."""
    from contextlib import ExitStack
    import concourse.bass as bass
    import concourse.mybir as mybir

    f32 = mybir.dt.float32
    f16 = mybir.dt.float16
    bf16 = mybir.dt.bfloat16
    K = 6 * PACK
    LOFS = NG * G_TILE          # rhs column offset inside the packed input

    nc = bass.Bass()
    inp_d = nc.declare_dram_parameter("inp", [K, LOFS + NG * GW], bf16,
                                      isOutput=False)
    dp_d = nc.declare_dram_parameter("dp0", [G_TILE, NG * GW], f16,
                                     isOutput=True)

    # copy/DMA units: pairs of groups (last unit may be a single group).
    # Vector owns even units, Scalar odd units (so Scalar, whose queue
    # also carries an HWDGE ring, handles the final unit + its DMA).
    units = [(g, min(g + 1, NG - 1)) for g in range(0, NG, 2)]
    s_units = [u for u in range(len(units)) if u % 2 == 1]
    v_units = [u for u in range(len(units)) if u % 2 == 0]

    # rhs input chunks on the sync queue: [0,3), [3,7), [7,NG)
    cut1, cut2 = min(3, NG), min(7, NG)

    def in_need(g):
        if g < cut1:
            return 16
        if g < cut2:
            return 32
        return 48

    with ExitStack() as ctx:
        inp_s = ctx.enter_context(
            nc.sbuf_tensor("inp_s", [K, LOFS + NG * GW], bf16))
        ring = ctx.enter_context(
            nc.sbuf_tensor("ring", [G_TILE, NG * GW], f16))
        pt = ctx.enter_context(nc.psum_tensor("pt", [G_TILE, 4096], f32))

        lh_sem = ctx.enter_context(nc.semaphore("lh_sem"))
        in_sem = ctx.enter_context(nc.semaphore("in_sem"))
        pe_sem = ctx.enter_context(nc.semaphore("pe_sem"))
        sc_sem = ctx.enter_context(nc.semaphore("sc_sem"))
        vc_sem = ctx.enter_context(nc.semaphore("vc_sem"))
        od_sem = ctx.enter_context(nc.semaphore("od_sem"))
        block = ctx.enter_context(nc.Block())

        def wait_unit(eng, u):
            if u in s_units:
                eng.wait_ge(sc_sem, s_units.index(u) + 1)
            else:
                eng.wait_ge(vc_sem, v_units.index(u) + 1)

        @block.sync
        def _(sync):
            sync.dma_start(inp_s[:, LOFS:LOFS + cut1 * GW],
                           inp_d[:, LOFS:LOFS + cut1 * GW],
                           ).then_inc(in_sem, 16)
            sync.dma_start(inp_s[:, LOFS + cut1 * GW:LOFS + cut2 * GW],
                           inp_d[:, LOFS + cut1 * GW:LOFS + cut2 * GW],
                           ).then_inc(in_sem, 16)
            sync.dma_start(inp_s[:, LOFS + cut2 * GW:],
                           inp_d[:, LOFS + cut2 * GW:]).then_inc(in_sem, 16)
            for u in v_units:
                g0, g1 = units[u]
                wait_unit(sync, u)
                sync.dma_start(dp_d[:, g0 * GW:(g1 + 1) * GW],
                               ring[:, g0 * GW:(g1 + 1) * GW],
                               ).then_inc(od_sem, 16)

        @block.tensor
        def _(tensor):
            # HAM warm-up: dummy matmuls on stale SBUF into PSUM bank 7
            for _i in range(ND_DUMMY):
                nc.tensor.matmul(pt[:, 7 * GW:8 * GW],
                                 inp_s[:, 0:G_TILE], inp_s[:, 0:GW],
                                 start=True, stop=True)
            tensor.wait_ge(lh_sem, 16)
            cur_need = 0
            for g in range(NG):
                need = in_need(g)
                if need > cur_need:
                    tensor.wait_ge(in_sem, need)
                    cur_need = need
                if g >= 8:
                    wait_unit(tensor, (g - 8) // 2)
                b = g % 8
                nc.tensor.matmul(
                    pt[:, b * GW:(b + 1) * GW],
                    inp_s[:, g * G_TILE:(g + 1) * G_TILE],
                    inp_s[:, LOFS + g * GW:LOFS + (g + 1) * GW],
                    start=True, stop=True,
                ).then_inc(pe_sem, 1)

        @block.scalar
        def _(scalar):
            # trigger the activation-table load during the input dead
            # time, and carry the lhs input DMA (also warms this
            # queue's HWDGE ring for the self-issued output DMAs below)
            nc.scalar.activation(ring[0:1, 0:8], ring[0:1, 8:16],
                                 mybir.ActivationFunctionType.Copy, scale=1.0)
            nc.scalar.dma_start(inp_s[:, 0:LOFS],
                                inp_d[:, 0:LOFS]).then_inc(lh_sem, 16)
            for u in s_units:
                g0, g1 = units[u]
                scalar.wait_ge(pe_sem, g1 + 1)
                nc.scalar.activation(
                    ring[:, g0 * GW:(g1 + 1) * GW],
                    pt[:, (g0 % 8) * GW:(g1 % 8 + 1) * GW],
                    mybir.ActivationFunctionType.Copy, scale=1.0,
                ).then_inc(sc_sem, 1)
                nc.scalar.dma_start(dp_d[:, g0 * GW:(g1 + 1) * GW],
                                    ring[:, g0 * GW:(g1 + 1) * GW],
                                    ).then_inc(od_sem, 16)

        @block.vector
        def _(vector):
            for u in v_units:
                g0, g1 = units[u]
                vector.wait_ge(pe_sem, g1 + 1)
                nc.vector.tensor_copy(
                    ring[:, g0 * GW:(g1 + 1) * GW],
                    pt[:, (g0 % 8) * GW:(g1 % 8 + 1) * GW],
                ).then_inc(vc_sem, 1)

    return nc


RUN_OPTS = {}
LAST_RES = None
LAST_INFO = {}


def kernel(gth, pred):
    from concourse.bass_utils import run_bass_kernel_spmd
    import ml_dtypes

    gth = np.asarray(gth, np.float32).reshape(BC, H, W_IMG)
    pred = np.asarray(pred, np.float32).reshape(BC, H, W_IMG)

    gedge = _edge_maps(gth)
    pedge = _edge_maps(pred)

    pts = []
    for i in range(BC):
        gy, gx = np.nonzero(gedge[i])
        py, px = np.nonzero(pedge[i])
        pts.append((gy.astype(np.int64), gx.astype(np.int64),
                    py.astype(np.int64), px.astype(np.int64)))

    pair_tiles, pair_reqs = [], []
    for i in range(BC):
        gy, gx, py, px = pts[i]
        n_g, n_p = len(gy), len(py)
        if n_g and n_p:
            u_g = _nn_upper_bound(_edt_full(pedge[i]), gy, gx)
            v_p = _nn_upper_bound(_edt_full(gedge[i]), py, px)
            T_i = max(1, -(-n_g // G_TILE))
            tiles = _kd_tiles(gy, gx, T_i)
            reqs = _tile_reqs(tiles, gy, gx, py, px, u_g, v_p)
        else:
            tiles, reqs = [], []
        pair_tiles.append(tiles)
        pair_reqs.append(reqs)

    raw = [sum(len(r) for r in pair_reqs[i]) for i in range(BC)]
    order = sorted(range(BC), key=lambda i: -raw[i])
    assign = [[order[c], order[BC - 1 - c]] for c in range(N_CORES)]

    # Per core: flat column stream of (pair01, tile, cand-slice) cut at
    # 512-col group boundaries.
    core_groups = []   # per core: per group: list of (p01,t,cand,ofs)
    for c in range(N_CORES):
        groups, cur, used = [], [], 0
        for p01 in (0, 1):
            i = assign[c][p01]
            for t, r in enumerate(pair_reqs[i]):
                pos = 0
                while pos < len(r):
                    take = min(GW - used, len(r) - pos)
                    cur.append((p01, t, r[pos:pos + take], used))
                    used += take
                    pos += take
                    if used == GW:
                        groups.append(cur)
                        cur, used = [], 0
        if cur:
            groups.append(cur)
        core_groups.append(groups)

    NG = max(1, max(len(g) for g in core_groups))
    PACK = max(2, max((len(seglist) for groups in core_groups
                       for seglist in groups), default=2))
    K = 6 * PACK

    nc = _build_program(NG, PACK)

    LOFS = NG * G_TILE
    in_maps = []
    for c in range(N_CORES):
        inp = np.zeros((K, LOFS + NG * GW), np.float32)
        for g, seglist in enumerate(core_groups[c]):
            for s, (p01, t, cand, ofs) in enumerate(seglist):
                i = assign[c][p01]
                gy, gx, py, px = pts[i]
                rows = pair_tiles[i][t]
                cyg = np.full(G_TILE, SENTC, np.float32)
                cxg = np.full(G_TILE, SENTC, np.float32)
                cyg[:len(rows)] = gy[rows] - 128.0
                cxg[:len(rows)] = gx[rows] - 128.0
                inp[6 * s:6 * s + 6, g * G_TILE:(g + 1) * G_TILE] = \
                    _aug_g(cyg, cxg)
                inp[6 * s:6 * s + 6,
                    LOFS + g * GW + ofs:LOFS + g * GW + ofs + len(cand)] = \
                    _aug_p(py[cand] - 128.0, px[cand] - 128.0)
        in_maps.append({"inp": inp.astype(ml_dtypes.bfloat16)})

    res = run_bass_kernel_spmd(nc, in_maps, list(range(N_CORES)), **RUN_OPTS)
    global LAST_RES, LAST_INFO
    LAST_RES = res
    LAST_INFO = {"NG": NG, "PACK": PACK, "assign": assign}
    results = res.results

    losses = np.full(BC, np.nan, np.float64)
    for c in range(N_CORES):
        dp_raw = np.asarray(results[c]["dp0"], np.float32)
        colmax = dp_raw.max(axis=0)
        val_g = [None, None]
        dpv = [None, None]
        for p01 in (0, 1):
            i = assign[c][p01]
            nt = len(pair_tiles[i])
            val_g[p01] = np.full((max(nt, 1), G_TILE), -np.inf, np.float32)
            dpv[p01] = np.full(max(len(pts[i][2]), 1), -np.inf, np.float32)
        for g, seglist in enumerate(core_groups[c]):
            for (p01, t, cand, ofs) in seglist:
                c0 = g * GW + ofs
                blk = dp_raw[:, c0:c0 + len(cand)].max(axis=1)
                val_g[p01][t] = np.maximum(val_g[p01][t], blk)
                np.maximum.at(dpv[p01], cand, colmax[c0:c0 + len(cand)])
        for p01 in (0, 1):
            i = assign[c][p01]
            gy, gx, py, px = pts[i]
            n_g, n_p = len(gy), len(py)
            if n_g == 0 or n_p == 0:
                # reference yields nan whenever either set is empty
                losses[i] = np.nan
                continue
            tiles = pair_tiles[i]
            dgv = np.empty(n_g, np.float32)
            for t in range(len(tiles)):
                rows = tiles[t]
                dgv[rows] = val_g[p01][t, :len(rows)]
            d_g = np.sqrt(np.maximum(-4.0 * dgv.astype(np.float64), 0.0))
            d_p = np.sqrt(np.maximum(
                -4.0 * dpv[p01][:n_p].astype(np.float64), 0.0))
            losses[i] = _loss_from_nn(d_g, d_p, n_g, n_p)

    return np.float32(np.nanmean(losses.astype(np.float32)))
